# revision 2
# baseline (speedup 1.0000x reference)
"""Cross-modality attention Trainium2 kernel (8 NeuronCores, SPMD).

Per core (batch b = r//4, query slice s = r%4 of 576 tokens), all 9 (i,j)
attention pairs are computed with fp8 DoubleRow matmuls (contraction 256 per
pass):

- scores^T[k,q] = X_j^T (Wq^T Wk/sqrt(C)) X_i: stationary X_j key-chunk fp8,
  moving Qg_i fp8 (x256 scaled; exp() applies 1/256). V^T is produced by two
  extra matmuls per key chunk reusing the same stationary X_j.
- exp() on ACT writes P^T tiles directly in fp8.
- PV is V-stationary: ctx^T[c,q] = sum_k V^T[k,c] P^T[k,q], accumulated over
  9 256-key chunks in PSUM; q split (256,256,64) to pack PSUM banks.
- softmax denominator: matmul with a stationary of 128 replicated
  VS-valued columns -> denominator already broadcast across partitions
  (and pre-scaled by the V quantization factor).
- per-pair normalize+accumulate on DVE: ctx_i += U * (1/denom), f32 in SBUF.
- gate/out/Qg projections stay bf16 (precision); all biases are zero (spec).

Emission is software-pipelined: the scores+exp stream of pair p interleaves
with the PV stream of pair p-1 at key-chunk granularity so the PE stays busy
while ACT (exp, the co-bottleneck at ~120us) catches up.
"""

import os
from contextlib import ExitStack

import ml_dtypes
import numpy as np

import concourse.bass as bass
import concourse.tile as tile
from concourse import bacc, mybir
from concourse.bass_utils import run_bass_kernel_spmd

B, C, H, W = 2, 256, 48, 48
N = H * W            # 2304 tokens
NCORES = 8
NSLICE = 4
QS = N // NSLICE     # 576 query tokens per core
KC = N // 128        # 18 key chunks of 128
KC2 = N // 256       # 9 key chunks of 256 (DoubleRow)
QH = 288             # scores free-dim half (psum bank half)
VS = 32.0            # V fp8 quantization scale (folded into denominator)
QGS = 256.0          # Qg fp8 scale (exp applies 1/QGS)

F32 = mybir.dt.float32
BF16 = mybir.dt.bfloat16
FP8 = mybir.dt.float8e4
AF = mybir.ActivationFunctionType
ALU = mybir.AluOpType
DR = mybir.MatmulPerfMode.DoubleRow
DRS = mybir.MatmulPerfMode.DoubleRowSwInterleave

XCH = 3              # x8 load chunks of 768 tokens (6 kc each)
DEBUG_DUMP = bool(os.environ.get("K2_DEBUG"))


def _emit(ctx: ExitStack, tc: tile.TileContext, io: dict):
    nc = tc.nc
    P = 128

    persist = ctx.enter_context(tc.tile_pool(name="persist", bufs=1))
    pt_pool = ctx.enter_context(tc.tile_pool(name="pt", bufs=3))
    vst_pool = ctx.enter_context(tc.tile_pool(name="vst", bufs=2))
    rp_pool = ctx.enter_context(tc.tile_pool(name="rp", bufs=2))
    tmp_pool = ctx.enter_context(tc.tile_pool(name="tmp", bufs=2))
    fus_pool = ctx.enter_context(tc.tile_pool(name="fus", bufs=2))
    osb_pool = ctx.enter_context(tc.tile_pool(name="osb", bufs=2))
    ps_s = ctx.enter_context(tc.tile_pool(name="ps_s", bufs=2, space="PSUM"))
    ps_u0 = ctx.enter_context(tc.tile_pool(name="ps_u0", bufs=1, space="PSUM"))
    ps_u1 = ctx.enter_context(tc.tile_pool(name="ps_u1", bufs=1, space="PSUM"))
    ps_d = ctx.enter_context(tc.tile_pool(name="ps_d", bufs=1, space="PSUM"))
    ps_m = ctx.enter_context(tc.tile_pool(name="ps_m", bufs=1, space="PSUM"))

    # ---- persistent tiles -----------------------------------------------
    prime = persist.tile([1, 4], F32, tag="prime")
    # x8 is only ever a (SwInterleave) stationary: host stores each 128-key
    # chunk pair-interleaved [t0 k127, t1 k127, t0 k126, ...] so LDWEIGHTS
    # reads contiguously (FWL) instead of DoubleRow's slow interleaved read.
    xs = [persist.tile([P, KC, 2 * P], FP8, tag=f"xs{j}", name=f"xs{j}")
          for j in range(3)]
    xq = [persist.tile([P, 2, QS], BF16, tag=f"xq{i}", name=f"xq{i}")
          for i in range(3)]
    qg8 = [persist.tile([P, 2, QS], FP8, tag=f"qg8{i}", name=f"qg8{i}")
           for i in range(3)]
    gate = [persist.tile([P, 2, QS], BF16, tag=f"gate{i}", name=f"gate{i}")
            for i in range(3)]
    ctxa = [persist.tile([P, 2, QS], BF16, tag=f"ctx{i}", name=f"ctx{i}")
            for i in range(3)]
    wsb = {}
    for wn in ("m_t", "gate_wt", "out_wt"):
        wsb[wn] = persist.tile([P, 2, C], BF16, tag=f"w_{wn}", name=f"w_{wn}")
    wsb["v_wt8"] = persist.tile([P, 2, C], FP8, tag="w_v", name="w_v")
    wsb["v_lo8"] = persist.tile([P, 2, C], FP8, tag="w_vlo", name="w_vlo")
    ones_f = persist.tile([P, C], F32, tag="ones_f")
    ones8 = persist.tile([P, 2, P], FP8, tag="ones8")

    # ---- schedule: prologue ---------------------------------------------
    # Prime the exp table set (also contains Tanh) off the critical path.
    nc.vector.memset(prime[:, :], 0.0)
    nc.scalar.activation(out=prime[:, :], in_=prime[:, :], func=AF.Exp,
                         scale=0.0)

    nc.sync.dma_start(out=wsb["m_t"][:, :, :], in_=io["m_t"][:, :, :])
    for i in range(3):
        nc.sync.dma_start(out=xq[i][:, :, :], in_=io[f"xq{i}"][:, :, :])
    nc.sync.dma_start(out=wsb["gate_wt"][:, :, :], in_=io["gate_wt"][:, :, :])
    # bulk x8 loads on the gpsimd queue, chunked so scores start early
    for j in range(3):
        for xc in range(XCH):
            sl = slice(xc * 6, (xc + 1) * 6)
            nc.gpsimd.dma_start(out=xs[j][:, sl, :], in_=io[f"x8{j}"][:, sl, :])
    nc.sync.dma_start(out=wsb["v_wt8"][:, :, :], in_=io["v_wt8"][:, :, :])
    nc.sync.dma_start(out=wsb["v_lo8"][:, :, :], in_=io["v_lo8"][:, :, :])
    nc.sync.dma_start(out=wsb["out_wt"][:, :, :], in_=io["out_wt"][:, :, :])

    nc.vector.memset(ones_f[:, :], VS)
    nc.vector.tensor_copy(ones8.rearrange("p a b -> p (a b)"), ones_f[:, :])

    def proj_ps():
        return ps_s.tile([P, 2, 512], F32, tag="s", name="ps")

    def project_qg(i):
        # Qg = m_t^T Xq (bf16), cast to fp8 with x256 scale.
        for u in range(2):
            ps = proj_ps()
            for qh in range(2):
                for t in range(2):
                    nc.tensor.matmul(
                        ps[:, qh, 0:QH], wsb["m_t"][:, t, u * P:(u + 1) * P],
                        xq[i][:, t, qh * QH:(qh + 1) * QH],
                        start=(t == 0), stop=(t == 1), skip_group_check=True)
            nc.vector.tensor_scalar_mul(qg8[i][:, u, :],
                                        ps[:, :, 0:QH], QGS)

    def project_gate_unit(i, u):
        # gate = sigmoid(Wg x) = 0.5*tanh(z/2)+0.5; tanh shares exp's table.
        ps = proj_ps()
        for qh in range(2):
            for t in range(2):
                nc.tensor.matmul(
                    ps[:, qh, 0:QH], wsb["gate_wt"][:, t, u * P:(u + 1) * P],
                    xq[i][:, t, qh * QH:(qh + 1) * QH],
                    start=(t == 0), stop=(t == 1), skip_group_check=True)
        nc.scalar.activation(out=gate[i][:, u, :], in_=ps[:, :, 0:QH],
                             func=AF.Tanh, scale=0.5)
        nc.vector.tensor_scalar(out=gate[i][:, u, :], in0=gate[i][:, u, :],
                                scalar1=0.5, scalar2=0.5,
                                op0=ALU.mult, op1=ALU.add)

    gate_units = [(i, u) for i in range(3) for u in range(2)]

    for i in range(3):
        project_qg(i)
    if DEBUG_DUMP:
        for i in range(3):
            nc.sync.dma_start(out=io["dbg_qg"][i], in_=qg8[i][:, :, :])
            nc.sync.dma_start(out=io["dbg_gate"][i], in_=gate[i][:, :, :])

    # ---- attention pipeline ---------------------------------------------
    pairs = [(j, i) for j in range(3) for i in range(3)]
    pt_t = {}
    vst_t = {}
    state = {}
    rps = {}
    ucs = {}

    def s_unit(p, kc):
        # scores for pair p at key chunk kc (+ V projection when i == 0).
        # One stationary (the X_j key chunk) serves every matmul in the
        # unit, keeping the LDWEIGHTS stream under the matmul stream.
        # psum flat: [0:512] scores (bank0); [512:576] scores tail +
        # [576:832] V^T (bank1), V hi+lo accumulating onto the tail's
        # bank zero-init.  bank1 first so exp only waits on the 512-matmul.
        j, i = p
        ps = ps_s.tile([P, 2, 512], F32, tag="s")
        fl = ps.rearrange("p a b -> p (a b)")
        lhs = xs[j][:, kc, :].rearrange("p (a b) -> p a b", a=2)
        nc.tensor.matmul(fl[:, 512:QS], lhs, qg8[i][:, :, 512:QS],
                         start=True, stop=True, perf_mode=DRS,
                         skip_group_check=True)
        if i == 0:
            nc.tensor.matmul(fl[:, QS:QS + C], lhs, wsb["v_wt8"][:, :, :],
                             start=False, stop=False, perf_mode=DRS,
                             skip_group_check=True)
            nc.tensor.matmul(fl[:, QS:QS + C], lhs, wsb["v_lo8"][:, :, :],
                             start=False, stop=True, perf_mode=DRS,
                             skip_group_check=True)
        nc.tensor.matmul(fl[:, 0:512], lhs, qg8[i][:, :, 0:512],
                         start=True, stop=True, perf_mode=DRS,
                         skip_group_check=True)
        kc2, plane = divmod(kc, 2)
        nc.scalar.activation(out=pt_t[p][:, kc2, plane, :],
                             in_=fl[:, 0:QS], func=AF.Exp,
                             scale=1.0 / QGS)
        if i == 0:
            nc.vector.tensor_copy(vst_t[j][:, kc2, plane, :],
                                  fl[:, QS:QS + C])

    def u_unit(p, kc2):
        # PV matmuls for pair p at 256-key chunk kc2.
        # One start=True per PSUM bank: start pends the WHOLE 2KiB bank, so
        # sibling accumulation groups in the m bank must ride the first
        # group's zero-init and accumulate with start=False from kc2 == 0.
        j, i = p
        u0, u1, d, m = state[p]
        start, stop = kc2 == 0, kc2 == KC2 - 1
        pt = pt_t[p]
        mv = pt[:, kc2, :, 0:512]
        mv_tl = pt[:, kc2, :, 512:QS]
        for ch, ups in ((0, u0), (1, u1)):
            lhs = vst_t[j][:, kc2, :, ch * P:(ch + 1) * P]
            nc.tensor.matmul(ups[:, :], lhs, mv, start=start, stop=stop,
                             perf_mode=DR, skip_group_check=True)
            nc.tensor.matmul(m[:, ch * 64:(ch + 1) * 64], lhs, mv_tl,
                             start=(start and ch == 0), stop=stop,
                             perf_mode=DR, skip_group_check=True)

    def d_unit(p, kc2):
        # denominator matmuls (constant replicated-VS stationary)
        j, i = p
        u0, u1, d, m = state[p]
        start, stop = kc2 == 0, kc2 == KC2 - 1
        pt = pt_t[p]
        # ones8 is constant so its interleaved layout is itself: SwInterleave
        # gives these matmuls the fast contiguous weight load too.
        nc.tensor.matmul(d[:, :], ones8, pt[:, kc2, :, 0:512],
                         start=start, stop=stop, perf_mode=DRS,
                         skip_group_check=True)
        nc.tensor.matmul(m[:, 128:192], ones8, pt[:, kc2, :, 512:QS],
                         start=False, stop=stop, perf_mode=DRS,
                         skip_group_check=True)

    def recip_pair(p):
        # The main reciprocal runs early (the denominator chain finishes
        # mid-slot) so the products at the pair boundary free the U banks
        # with ~no PE stall.  The 64-tail reciprocal waits for the boundary:
        # reading the m bank mid-slot would serialize against the still-
        # accumulating U tail matmuls in the same bank.
        u0, u1, d, m = state[p]
        rp = rp_pool.tile([P, QS], F32, tag="rp")
        nc.vector.reciprocal(rp[:, 0:512], d[:, :])
        rps[p] = rp

    def normalize_free(p):
        # copy U psum banks to SBUF so they free with only copy latency
        u0, u1, d, m = state[p]
        uc = tmp_pool.tile([P, 2, QS], F32, tag="ucpy", name="uc")
        nc.vector.tensor_copy(uc[:, 0, 0:512], u0[:, :])
        nc.vector.tensor_copy(uc[:, 1, 0:512], u1[:, :])
        ucs[p] = uc

    def normalize_free_m(p):
        # the m-bank copies run after the next slot's first vst cast so the
        # psum rotation is not blocked behind them on the in-order DVE
        u0, u1, d, m = state[p]
        uc = ucs[p]
        dtl = tmp_pool.tile([P, 64], F32, tag="dtl", name="dtl")
        nc.vector.tensor_copy(uc[:, :, 512:QS],
                              m[:, 0:128].rearrange("p (a b) -> p a b", a=2))
        nc.vector.tensor_copy(dtl[:, :], m[:, 128:192])
        ucs[p] = (uc, dtl)

    def normalize_products(p, first):
        # ctx_i (+)= U * (1/denom); denom rows already broadcast and carry
        # the VS factor from the ones8 stationary.
        j, i = p
        uc, dtl = ucs[p]
        rp = rps[p]
        nc.vector.reciprocal(rp[:, 512:QS], dtl[:, :])
        if first:
            dst = ctxa[i]
        else:
            dst = tmp_pool.tile([P, 2, QS], BF16, tag="tmp")
        for ch in range(2):
            nc.vector.tensor_mul(dst[:, ch, 0:512], uc[:, ch, 0:512],
                                 rp[:, 0:512])
            nc.vector.tensor_mul(dst[:, ch, 512:QS], uc[:, ch, 512:QS],
                                 rp[:, 512:QS])
        if not first:
            nc.vector.tensor_add(ctxa[i][:, :, :], ctxa[i][:, :, :],
                                 dst[:, :, :])

    def finish_blend(i):
        # fused = x + gate*(ctx-x)
        fus = fus_pool.tile([P, 2, QS], BF16, tag="fus", name=f"fus{i}")
        for t in range(2):
            diff = tmp_pool.tile([P, QS], BF16, tag="tmpd")
            nc.vector.tensor_sub(diff[:, :], ctxa[i][:, t, :], xq[i][:, t, :])
            nc.vector.tensor_mul(diff[:, :], diff[:, :], gate[i][:, t, :])
            nc.vector.tensor_add(fus[:, t, :], diff[:, :], xq[i][:, t, :])
        fus_t[i] = fus

    def finish_blend_pre(i):
        # h = x + gate*(ctx_partial - x) for the LAST pair's modality:
        # precomputed before the last pair lands so the tail only needs
        # fused = h + gate*(U*rp).
        fus = fus_pool.tile([P, 2, QS], BF16, tag="fus", name=f"fus{i}")
        for t in range(2):
            diff = tmp_pool.tile([P, QS], BF16, tag="tmpd")
            nc.vector.tensor_sub(diff[:, :], ctxa[i][:, t, :], xq[i][:, t, :])
            nc.vector.tensor_mul(diff[:, :], diff[:, :], gate[i][:, t, :])
            nc.vector.tensor_add(fus[:, t, :], diff[:, :], xq[i][:, t, :])
        fus_t[i] = fus

    def finish_last(p):
        # tail of the final pair: fused = h + gate*(U*rp), reading U psum
        # directly (no bank-freeing urgency at the end) and split by query
        # half so the output projection starts after ~1us of DVE work.
        j, i = p
        u0, u1, d, m = state[p]
        rp = rps[p]
        fus = fus_t[i]
        e = tmp_pool.tile([P, 2, QS], BF16, tag="tmp", name="e")
        for ch, ups in ((0, u0), (1, u1)):
            nc.vector.tensor_mul(e[:, ch, 0:QH], ups[:, 0:QH], rp[:, 0:QH])
        nc.vector.tensor_mul(e[:, :, 0:QH], e[:, :, 0:QH],
                             gate[i][:, :, 0:QH])
        nc.vector.tensor_add(fus[:, :, 0:QH], fus[:, :, 0:QH], e[:, :, 0:QH])
        for u in range(2):
            finish_out_qh(i, u, 0)
        nc.vector.reciprocal(rp[:, 512:QS], m[:, 128:192])
        for ch, ups in ((0, u0), (1, u1)):
            nc.vector.tensor_mul(e[:, ch, QH:512], ups[:, QH:512],
                                 rp[:, QH:512])
            nc.vector.tensor_mul(e[:, ch, 512:QS],
                                 m[:, ch * 64:(ch + 1) * 64], rp[:, 512:QS])
        nc.vector.tensor_mul(e[:, :, QH:QS], e[:, :, QH:QS],
                             gate[i][:, :, QH:QS])
        nc.vector.tensor_add(fus[:, :, QH:QS], fus[:, :, QH:QS],
                             e[:, :, QH:QS])
        for u in range(2):
            finish_out_qh(i, u, 1)

    def finish_out_qh(i, u, qh):
        fus = fus_t[i]
        po = proj_ps()
        for t in range(2):
            nc.tensor.matmul(
                po[:, 0, 0:QH], wsb["out_wt"][:, t, u * P:(u + 1) * P],
                fus[:, t, qh * QH:(qh + 1) * QH],
                start=(t == 0), stop=(t == 1), skip_group_check=True)
        osb = osb_pool.tile([P, QH], F32, tag="osbq")
        nc.vector.tensor_copy(osb[:, :], po[:, 0, 0:QH])
        q = nc.sync if qh == 0 else nc.gpsimd
        q.dma_start(out=io["out"][i, u * P:(u + 1) * P,
                                  qh * QH:(qh + 1) * QH], in_=osb[:, :])

    def finish_out(i, u):
        # out = Wout fused; copy + 2-way chunked DMA out
        fus = fus_t[i]
        po = proj_ps()
        for qh in range(2):
            for t in range(2):
                nc.tensor.matmul(
                    po[:, qh, 0:QH], wsb["out_wt"][:, t, u * P:(u + 1) * P],
                    fus[:, t, qh * QH:(qh + 1) * QH],
                    start=(t == 0), stop=(t == 1), skip_group_check=True)
        osb = osb_pool.tile([P, QS], F32, tag="osb")
        nc.vector.tensor_copy(osb[:, :], po[:, :, 0:QH])
        nc.sync.dma_start(out=io["out"][i, u * P:(u + 1) * P, 0:QH],
                          in_=osb[:, 0:QH])
        nc.gpsimd.dma_start(out=io["out"][i, u * P:(u + 1) * P, QH:QS],
                            in_=osb[:, QH:QS])

    fus_t = {}

    NP = len(pairs)
    LASTI = pairs[NP - 1][1]

    def emit_gate_filler(k):
        if k < len(gate_units):
            gi, gu = gate_units[k]
            project_gate_unit(gi, gu)

    def alloc_state(p):
        state[p] = (
            ps_u0.tile([P, 512], F32, tag="u0", name="u0"),
            ps_u1.tile([P, 512], F32, tag="u1", name="u1"),
            ps_d.tile([P, 512], F32, tag="d", name="d"),
            ps_m.tile([P, 512], F32, tag="m", name="m"),
        )

    for slot in range(NP):
        p_s = pairs[slot]                                  # scores phase
        p_v = pairs[slot - 1] if slot >= 1 else None       # PV phase
        p_n = pairs[slot - 2] if slot >= 2 else None       # deferred norm
        j, i = p_s
        pt_t[p_s] = pt_pool.tile([P, KC2, 2, QS], FP8, tag="pt",
                                 name=f"pt{j}{i}")
        if i == 0:
            vst_t[j] = vst_pool.tile([P, KC2, 2, C], FP8, tag="vst",
                                     name=f"vst{j}")
        for mtick in range(KC2):
            s_unit(p_s, 2 * mtick)
            if p_n is not None and mtick == 0:
                normalize_free_m(p_n)
            if p_v is not None and mtick >= 1:
                if mtick == 1:
                    alloc_state(p_v)
                u_unit(p_v, mtick - 1)
                if mtick <= 4:
                    d_unit(p_v, 2 * (mtick - 1))
            s_unit(p_s, 2 * mtick + 1)
            if slot == 0:
                emit_gate_filler(mtick)
            if p_n is not None and mtick == 2:
                normalize_products(p_n, first=(p_n[0] == 0))
                if DEBUG_DUMP:
                    nc.sync.dma_start(
                        out=io["dbg_ctx"][p_n[0] * 3 + p_n[1]],
                        in_=ctxa[p_n[1]][:, :, :])
            if p_n is not None and p_n[0] == 2:
                if mtick == 3:
                    finish_blend(p_n[1])
                elif mtick == 5:
                    finish_out(p_n[1], 0)
                elif mtick == 6:
                    finish_out(p_n[1], 1)
            if p_v is not None:
                if mtick <= 4 and mtick >= 1:
                    d_unit(p_v, 2 * mtick - 1)
                elif mtick == 5:
                    d_unit(p_v, KC2 - 1)
                    recip_pair(p_v)
        if p_v is not None:
            u_unit(p_v, KC2 - 1)
            normalize_free(p_v)
        if DEBUG_DUMP and p_s == (0, 0):
            nc.sync.dma_start(out=io["dbg_pt"][:], in_=pt_t[p_s][:, :, :, :])
            nc.sync.dma_start(out=io["dbg_vst"][:], in_=vst_t[0][:, :, :, :])

    # ---- epilogue: PV of the last pair, hand-interleaved with the
    # previous pair's normalize/finish so the PE queue never blocks.
    p_v, p_n = pairs[NP - 1], pairs[NP - 2]
    alloc_state(p_v)
    normalize_free_m(p_n)
    for kc2 in range(3):
        u_unit(p_v, kc2)
        d_unit(p_v, 2 * kc2)
        d_unit(p_v, 2 * kc2 + 1)
    normalize_products(p_n, first=False)
    if DEBUG_DUMP:
        nc.sync.dma_start(out=io["dbg_ctx"][p_n[0] * 3 + p_n[1]],
                          in_=ctxa[p_n[1]][:, :, :])
    finish_blend(p_n[1])
    for kc2 in range(3, 6):
        u_unit(p_v, kc2)
        for dk in (2 * kc2, 2 * kc2 + 1):
            if dk < KC2:
                d_unit(p_v, dk)
    recip_pair(p_v)
    finish_blend_pre(LASTI)
    finish_out(p_n[1], 0)
    for kc2 in range(6, KC2):
        u_unit(p_v, kc2)
    finish_out(p_n[1], 1)
    finish_last(p_v)
    if DEBUG_DUMP:
        nc.sync.dma_start(out=io["dbg_ctx"][8], in_=ctxa[2][:, :, :])
    finish_out(LASTI, 0)
    finish_out(LASTI, 1)

def _build():
    nc = bacc.Bacc("TRN2", target_bir_lowering=False, debug=False,
                   num_devices=NCORES)
    io = {}
    for j in range(3):
        io[f"x8{j}"] = nc.declare_dram_parameter(f"x8{j}", [128, KC, 2 * 128],
                                                 FP8, isOutput=False)
        io[f"xq{j}"] = nc.declare_dram_parameter(f"xq{j}", [128, 2, QS], BF16,
                                                 isOutput=False)
    for wn, dt_ in (("m_t", BF16), ("v_wt8", FP8), ("v_lo8", FP8),
                    ("gate_wt", BF16), ("out_wt", BF16)):
        io[wn] = nc.declare_dram_parameter(wn, [128, 2, C], dt_, isOutput=False)
    io["out"] = nc.declare_dram_parameter("out", [3, C, QS], F32, isOutput=True)
    if DEBUG_DUMP:
        io["dbg_qg"] = nc.declare_dram_parameter(
            "dbg_qg", [3, 128, 2, QS], FP8, isOutput=True)
        io["dbg_gate"] = nc.declare_dram_parameter(
            "dbg_gate", [3, 128, 2, QS], BF16, isOutput=True)
        io["dbg_pt"] = nc.declare_dram_parameter(
            "dbg_pt", [128, KC2, 2, QS], FP8, isOutput=True)
        io["dbg_vst"] = nc.declare_dram_parameter(
            "dbg_vst", [128, KC2, 2, C], FP8, isOutput=True)
        io["dbg_ctx"] = nc.declare_dram_parameter(
            "dbg_ctx", [9, 128, 2, QS], F32, isOutput=True)

    with tile.TileContext(nc) as tc:
        with ExitStack() as ctx:
            _emit(ctx, tc, io)
    nc.compile()
    return nc


_CACHED_NC = None


def _get_nc():
    global _CACHED_NC
    if _CACHED_NC is None:
        _CACHED_NC = _build()
    return _CACHED_NC


def _prep_host(inputs: dict):
    f32 = np.float32
    bf16 = ml_dtypes.bfloat16
    f8 = ml_dtypes.float8_e4m3
    feats = [np.ascontiguousarray(inputs[f"feat{j}"], dtype=f32).reshape(B, C, N)
             for j in range(3)]
    for bn in ("q_b", "k_b", "v_b", "gate_b", "out_b"):
        if not np.all(np.asarray(inputs[bn]) == 0):
            raise NotImplementedError(f"{bn} != 0 unsupported (spec fill=zeros)")
    pos = np.asarray(inputs["pos_embedding"], f32).reshape(C, 1)
    scale = np.float32(C ** -0.5)

    def cpt(a, dt_):  # [C, X] -> [128, 2, X] with c = t*128 + p
        return np.ascontiguousarray(
            a.reshape(2, 128, a.shape[1]).transpose(1, 0, 2).astype(dt_))

    q_w = np.asarray(inputs["q_w"], f32)
    k_w = np.asarray(inputs["k_w"], f32)
    v_s = np.asarray(inputs["v_w"], f32).T * np.float32(VS)
    v_hi = v_s.astype(f8)
    shared = {
        "m_t": cpt((q_w.T @ k_w) * scale, bf16),
        "v_wt8": cpt(v_hi.astype(f32), f8),
        "v_lo8": cpt(v_s - v_hi.astype(f32), f8),
        "gate_wt": cpt(np.asarray(inputs["gate_w"], f32).T, bf16),
        "out_wt": cpt(np.asarray(inputs["out_w"], f32).T, bf16),
    }
    def interleave_x8(xall):
        # [C, N] -> [128, KC, 256] fp8: per 128-key chunk, pair-interleave
        # the two 128-channel planes with keys reversed (the SwInterleave
        # stationary layout; the matmul output key order comes out normal).
        a = xall.reshape(2, 128, KC, 128)        # [t, p, kc, k]
        a = a[:, :, :, ::-1].transpose(1, 2, 3, 0)   # [p, kc, k_rev, t]
        return np.ascontiguousarray(a.reshape(128, KC, 256).astype(f8))

    in_maps = []
    for r in range(NCORES):
        b, s = r // NSLICE, r % NSLICE
        im = dict(shared)
        for j in range(3):
            xall = feats[j][b] + pos
            im[f"x8{j}"] = interleave_x8(xall)
            im[f"xq{j}"] = cpt(xall[:, s * QS:(s + 1) * QS], bf16)
        in_maps.append(im)
    return in_maps


def _run(inputs: dict, trace: bool = False, tmpdir: str | None = None):
    in_maps = _prep_host(inputs)
    nc = _get_nc()
    res = run_bass_kernel_spmd(nc, in_maps, core_ids=list(range(NCORES)),
                               trace=trace, tmpdir=tmpdir)
    full = np.empty((3, B, C, N), dtype=np.float32)
    for r in range(NCORES):
        b, s = r // NSLICE, r % NSLICE
        full[:, b, :, s * QS:(s + 1) * QS] = res.results[r]["out"]
    full = full.reshape(3, B, C, H, W)
    return (full[0], full[1], full[2]), res


def kernel(**inputs):
    outs, _ = _run(inputs, trace=bool(os.environ.get("KERNEL_TRACE")))
    return outs


# revision 3
# speedup vs baseline: 1.0001x; 1.0001x over previous
"""Cross-modality attention Trainium2 kernel (8 NeuronCores, SPMD).

Per core (batch b = r//4, query slice s = r%4 of 576 tokens), all 9 (i,j)
attention pairs are computed with fp8 DoubleRow matmuls (contraction 256 per
pass):

- scores^T[k,q] = X_j^T (Wq^T Wk/sqrt(C)) X_i: stationary X_j key-chunk fp8,
  moving Qg_i fp8 (x256 scaled; exp() applies 1/256). V^T is produced by two
  extra matmuls per key chunk reusing the same stationary X_j.
- exp() on ACT writes P^T tiles directly in fp8.
- PV is V-stationary: ctx^T[c,q] = sum_k V^T[k,c] P^T[k,q], accumulated over
  9 256-key chunks in PSUM; q split (256,256,64) to pack PSUM banks.
- softmax denominator: matmul with a stationary of 128 replicated
  VS-valued columns -> denominator already broadcast across partitions
  (and pre-scaled by the V quantization factor).
- per-pair normalize+accumulate on DVE: ctx_i += U * (1/denom), f32 in SBUF.
- gate/out/Qg projections stay bf16 (precision); all biases are zero (spec).

Emission is software-pipelined: the scores+exp stream of pair p interleaves
with the PV stream of pair p-1 at key-chunk granularity so the PE stays busy
while ACT (exp, the co-bottleneck at ~120us) catches up.
"""

import os
from contextlib import ExitStack

import ml_dtypes
import numpy as np

import concourse.bass as bass
import concourse.tile as tile
from concourse import bacc, mybir
from concourse.bass_utils import run_bass_kernel_spmd

B, C, H, W = 2, 256, 48, 48
N = H * W            # 2304 tokens
NCORES = 8
NSLICE = 4
QS = N // NSLICE     # 576 query tokens per core
KC = N // 128        # 18 key chunks of 128
KC2 = N // 256       # 9 key chunks of 256 (DoubleRow)
QH = 288             # scores free-dim half (psum bank half)
VS = 32.0            # V fp8 quantization scale (folded into denominator)
QGS = 256.0          # Qg fp8 scale (exp applies 1/QGS)

F32 = mybir.dt.float32
BF16 = mybir.dt.bfloat16
FP8 = mybir.dt.float8e4
AF = mybir.ActivationFunctionType
ALU = mybir.AluOpType
DR = mybir.MatmulPerfMode.DoubleRow
DRS = mybir.MatmulPerfMode.DoubleRowSwInterleave

XCH = 3              # x8 load chunks of 768 tokens (6 kc each)
DEBUG_DUMP = bool(os.environ.get("K2_DEBUG"))


def _emit(ctx: ExitStack, tc: tile.TileContext, io: dict):
    nc = tc.nc
    P = 128

    persist = ctx.enter_context(tc.tile_pool(name="persist", bufs=1))
    pt_pool = ctx.enter_context(tc.tile_pool(name="pt", bufs=3))
    vst_pool = ctx.enter_context(tc.tile_pool(name="vst", bufs=2))
    rp_pool = ctx.enter_context(tc.tile_pool(name="rp", bufs=2))
    tmp_pool = ctx.enter_context(tc.tile_pool(name="tmp", bufs=2))
    fus_pool = ctx.enter_context(tc.tile_pool(name="fus", bufs=2))
    osb_pool = ctx.enter_context(tc.tile_pool(name="osb", bufs=2))
    ps_s = ctx.enter_context(tc.tile_pool(name="ps_s", bufs=2, space="PSUM"))
    ps_u0 = ctx.enter_context(tc.tile_pool(name="ps_u0", bufs=1, space="PSUM"))
    ps_u1 = ctx.enter_context(tc.tile_pool(name="ps_u1", bufs=1, space="PSUM"))
    ps_d = ctx.enter_context(tc.tile_pool(name="ps_d", bufs=1, space="PSUM"))
    ps_m = ctx.enter_context(tc.tile_pool(name="ps_m", bufs=1, space="PSUM"))

    # ---- persistent tiles -----------------------------------------------
    prime = persist.tile([1, 4], F32, tag="prime")
    # x8 is only ever a (SwInterleave) stationary: host stores each 128-key
    # chunk pair-interleaved [t0 k127, t1 k127, t0 k126, ...] so LDWEIGHTS
    # reads contiguously (FWL) instead of DoubleRow's slow interleaved read.
    xs = [persist.tile([P, KC, 2 * P], FP8, tag=f"xs{j}", name=f"xs{j}")
          for j in range(3)]
    xq = [persist.tile([P, 2, QS], BF16, tag=f"xq{i}", name=f"xq{i}")
          for i in range(3)]
    qg8 = [persist.tile([P, 2, QS], FP8, tag=f"qg8{i}", name=f"qg8{i}")
           for i in range(3)]
    gate = [persist.tile([P, 2, QS], BF16, tag=f"gate{i}", name=f"gate{i}")
            for i in range(3)]
    ctxa = [persist.tile([P, 2, QS], BF16, tag=f"ctx{i}", name=f"ctx{i}")
            for i in range(3)]
    wsb = {}
    for wn in ("m_t", "gate_wt", "out_wt"):
        wsb[wn] = persist.tile([P, 2, C], BF16, tag=f"w_{wn}", name=f"w_{wn}")
    wsb["v_wt8"] = persist.tile([P, 2, C], FP8, tag="w_v", name="w_v")
    wsb["v_lo8"] = persist.tile([P, 2, C], FP8, tag="w_vlo", name="w_vlo")
    ones_f = persist.tile([P, C], F32, tag="ones_f")
    ones8 = persist.tile([P, 2, P], FP8, tag="ones8")

    # ---- schedule: prologue ---------------------------------------------
    # Prime the exp table set (also contains Tanh) off the critical path.
    nc.vector.memset(prime[:, :], 0.0)
    nc.scalar.activation(out=prime[:, :], in_=prime[:, :], func=AF.Exp,
                         scale=0.0)

    nc.sync.dma_start(out=wsb["m_t"][:, :, :], in_=io["m_t"][:, :, :])
    for i in range(3):
        nc.sync.dma_start(out=xq[i][:, :, :], in_=io[f"xq{i}"][:, :, :])
    nc.sync.dma_start(out=wsb["gate_wt"][:, :, :], in_=io["gate_wt"][:, :, :])
    # x80 loads now (first scores need it); x81/x82 deferred below so they
    # don't contend with xq/weights for DMA engines during startup
    for xc in range(XCH):
        sl = slice(xc * 6, (xc + 1) * 6)
        nc.gpsimd.dma_start(out=xs[0][:, sl, :], in_=io["x80"][:, sl, :])
    nc.sync.dma_start(out=wsb["v_wt8"][:, :, :], in_=io["v_wt8"][:, :, :])
    nc.sync.dma_start(out=wsb["v_lo8"][:, :, :], in_=io["v_lo8"][:, :, :])
    nc.sync.dma_start(out=wsb["out_wt"][:, :, :], in_=io["out_wt"][:, :, :])

    nc.vector.memset(ones_f[:, :], VS)
    nc.vector.tensor_copy(ones8.rearrange("p a b -> p (a b)"), ones_f[:, :])

    def proj_ps():
        return ps_s.tile([P, 2, 512], F32, tag="s", name="ps")

    def project_qg(i):
        # Qg = m_t^T Xq (bf16), cast to fp8 with x256 scale.
        for u in range(2):
            ps = proj_ps()
            for qh in range(2):
                for t in range(2):
                    nc.tensor.matmul(
                        ps[:, qh, 0:QH], wsb["m_t"][:, t, u * P:(u + 1) * P],
                        xq[i][:, t, qh * QH:(qh + 1) * QH],
                        start=(t == 0), stop=(t == 1), skip_group_check=True)
            nc.vector.tensor_scalar_mul(qg8[i][:, u, :],
                                        ps[:, :, 0:QH], QGS)

    def project_gate_unit(i, u):
        # gate = sigmoid(Wg x) = 0.5*tanh(z/2)+0.5; tanh shares exp's table.
        ps = proj_ps()
        for qh in range(2):
            for t in range(2):
                nc.tensor.matmul(
                    ps[:, qh, 0:QH], wsb["gate_wt"][:, t, u * P:(u + 1) * P],
                    xq[i][:, t, qh * QH:(qh + 1) * QH],
                    start=(t == 0), stop=(t == 1), skip_group_check=True)
        nc.scalar.activation(out=gate[i][:, u, :], in_=ps[:, :, 0:QH],
                             func=AF.Tanh, scale=0.5)
        nc.vector.tensor_scalar(out=gate[i][:, u, :], in0=gate[i][:, u, :],
                                scalar1=0.5, scalar2=0.5,
                                op0=ALU.mult, op1=ALU.add)

    gate_units = [(i, u) for i in range(3) for u in range(2)]

    for i in range(3):
        project_qg(i)
    for j in (1, 2):
        for xc in range(XCH):
            sl = slice(xc * 6, (xc + 1) * 6)
            nc.gpsimd.dma_start(out=xs[j][:, sl, :], in_=io[f"x8{j}"][:, sl, :])
    if DEBUG_DUMP:
        for i in range(3):
            nc.sync.dma_start(out=io["dbg_qg"][i], in_=qg8[i][:, :, :])
            nc.sync.dma_start(out=io["dbg_gate"][i], in_=gate[i][:, :, :])

    # ---- attention pipeline ---------------------------------------------
    pairs = [(j, i) for j in range(3) for i in range(3)]
    pt_t = {}
    vst_t = {}
    state = {}
    rps = {}
    ucs = {}

    def s_unit(p, kc):
        # scores for pair p at key chunk kc (+ V projection when i == 0).
        # One stationary (the X_j key chunk) serves every matmul in the
        # unit, keeping the LDWEIGHTS stream under the matmul stream.
        # psum flat: [0:512] scores (bank0); [512:576] scores tail +
        # [576:832] V^T (bank1), V hi+lo accumulating onto the tail's
        # bank zero-init.  bank1 first so exp only waits on the 512-matmul.
        j, i = p
        ps = ps_s.tile([P, 2, 512], F32, tag="s")
        fl = ps.rearrange("p a b -> p (a b)")
        lhs = xs[j][:, kc, :].rearrange("p (a b) -> p a b", a=2)
        nc.tensor.matmul(fl[:, 512:QS], lhs, qg8[i][:, :, 512:QS],
                         start=True, stop=True, perf_mode=DRS,
                         skip_group_check=True)
        if i == 0:
            nc.tensor.matmul(fl[:, QS:QS + C], lhs, wsb["v_wt8"][:, :, :],
                             start=False, stop=False, perf_mode=DRS,
                             skip_group_check=True)
            nc.tensor.matmul(fl[:, QS:QS + C], lhs, wsb["v_lo8"][:, :, :],
                             start=False, stop=True, perf_mode=DRS,
                             skip_group_check=True)
        nc.tensor.matmul(fl[:, 0:512], lhs, qg8[i][:, :, 0:512],
                         start=True, stop=True, perf_mode=DRS,
                         skip_group_check=True)
        kc2, plane = divmod(kc, 2)
        nc.scalar.activation(out=pt_t[p][:, kc2, plane, :],
                             in_=fl[:, 0:QS], func=AF.Exp,
                             scale=1.0 / QGS)
        if i == 0:
            nc.vector.tensor_copy(vst_t[j][:, kc2, plane, :],
                                  fl[:, QS:QS + C])

    def u_unit(p, kc2):
        # PV matmuls for pair p at 256-key chunk kc2.
        # One start=True per PSUM bank: start pends the WHOLE 2KiB bank, so
        # sibling accumulation groups in the m bank must ride the first
        # group's zero-init and accumulate with start=False from kc2 == 0.
        j, i = p
        u0, u1, d, m = state[p]
        start, stop = kc2 == 0, kc2 == KC2 - 1
        pt = pt_t[p]
        mv = pt[:, kc2, :, 0:512]
        mv_tl = pt[:, kc2, :, 512:QS]
        for ch, ups in ((0, u0), (1, u1)):
            lhs = vst_t[j][:, kc2, :, ch * P:(ch + 1) * P]
            nc.tensor.matmul(ups[:, :], lhs, mv, start=start, stop=stop,
                             perf_mode=DR, skip_group_check=True)
            nc.tensor.matmul(m[:, ch * 64:(ch + 1) * 64], lhs, mv_tl,
                             start=(start and ch == 0), stop=stop,
                             perf_mode=DR, skip_group_check=True)

    def d_unit(p, kc2):
        # denominator matmuls (constant replicated-VS stationary)
        j, i = p
        u0, u1, d, m = state[p]
        start, stop = kc2 == 0, kc2 == KC2 - 1
        pt = pt_t[p]
        # ones8 is constant so its interleaved layout is itself: SwInterleave
        # gives these matmuls the fast contiguous weight load too.
        nc.tensor.matmul(d[:, :], ones8, pt[:, kc2, :, 0:512],
                         start=start, stop=stop, perf_mode=DRS,
                         skip_group_check=True)
        nc.tensor.matmul(m[:, 128:192], ones8, pt[:, kc2, :, 512:QS],
                         start=False, stop=stop, perf_mode=DRS,
                         skip_group_check=True)

    def recip_pair(p):
        # The main reciprocal runs early (the denominator chain finishes
        # mid-slot) so the products at the pair boundary free the U banks
        # with ~no PE stall.  The 64-tail reciprocal waits for the boundary:
        # reading the m bank mid-slot would serialize against the still-
        # accumulating U tail matmuls in the same bank.
        u0, u1, d, m = state[p]
        rp = rp_pool.tile([P, QS], F32, tag="rp")
        nc.vector.reciprocal(rp[:, 0:512], d[:, :])
        rps[p] = rp

    def normalize_free(p):
        # copy U psum banks to SBUF so they free with only copy latency
        u0, u1, d, m = state[p]
        uc = tmp_pool.tile([P, 2, QS], F32, tag="ucpy", name="uc")
        nc.scalar.copy(uc[:, 0, 0:512], u0[:, :])
        nc.scalar.copy(uc[:, 1, 0:512], u1[:, :])
        ucs[p] = uc

    def normalize_free_m(p):
        # the m-bank copies run after the next slot's first vst cast so the
        # psum rotation is not blocked behind them on the in-order DVE
        u0, u1, d, m = state[p]
        uc = ucs[p]
        dtl = tmp_pool.tile([P, 64], F32, tag="dtl", name="dtl")
        nc.vector.tensor_copy(uc[:, :, 512:QS],
                              m[:, 0:128].rearrange("p (a b) -> p a b", a=2))
        nc.vector.tensor_copy(dtl[:, :], m[:, 128:192])
        ucs[p] = (uc, dtl)

    def normalize_products(p, first):
        # ctx_i (+)= U * (1/denom); denom rows already broadcast and carry
        # the VS factor from the ones8 stationary.
        j, i = p
        uc, dtl = ucs[p]
        rp = rps[p]
        nc.vector.reciprocal(rp[:, 512:QS], dtl[:, :])
        if first:
            dst = ctxa[i]
        else:
            dst = tmp_pool.tile([P, 2, QS], BF16, tag="tmp")
        for ch in range(2):
            nc.vector.tensor_mul(dst[:, ch, 0:512], uc[:, ch, 0:512],
                                 rp[:, 0:512])
            nc.vector.tensor_mul(dst[:, ch, 512:QS], uc[:, ch, 512:QS],
                                 rp[:, 512:QS])
        if not first:
            nc.vector.tensor_add(ctxa[i][:, :, :], ctxa[i][:, :, :],
                                 dst[:, :, :])

    def finish_blend(i):
        # fused = x + gate*(ctx-x)
        fus = fus_pool.tile([P, 2, QS], BF16, tag="fus", name=f"fus{i}")
        for t in range(2):
            diff = tmp_pool.tile([P, QS], BF16, tag="tmpd")
            nc.vector.tensor_sub(diff[:, :], ctxa[i][:, t, :], xq[i][:, t, :])
            nc.vector.tensor_mul(diff[:, :], diff[:, :], gate[i][:, t, :])
            nc.vector.tensor_add(fus[:, t, :], diff[:, :], xq[i][:, t, :])
        fus_t[i] = fus

    def finish_blend_pre(i):
        # h = x + gate*(ctx_partial - x) for the LAST pair's modality:
        # precomputed before the last pair lands so the tail only needs
        # fused = h + gate*(U*rp).
        fus = fus_pool.tile([P, 2, QS], BF16, tag="fus", name=f"fus{i}")
        for t in range(2):
            diff = tmp_pool.tile([P, QS], BF16, tag="tmpd")
            nc.vector.tensor_sub(diff[:, :], ctxa[i][:, t, :], xq[i][:, t, :])
            nc.vector.tensor_mul(diff[:, :], diff[:, :], gate[i][:, t, :])
            nc.vector.tensor_add(fus[:, t, :], diff[:, :], xq[i][:, t, :])
        fus_t[i] = fus

    def finish_last(p):
        # tail of the final pair: fused = h + gate*(U*rp), reading U psum
        # directly (no bank-freeing urgency at the end) and split by query
        # half so the output projection starts after ~1us of DVE work.
        j, i = p
        u0, u1, d, m = state[p]
        rp = rps[p]
        fus = fus_t[i]
        e = tmp_pool.tile([P, 2, QS], BF16, tag="tmp", name="e")
        for ch, ups in ((0, u0), (1, u1)):
            nc.vector.tensor_mul(e[:, ch, 0:QH], ups[:, 0:QH], rp[:, 0:QH])
        nc.vector.tensor_mul(e[:, :, 0:QH], e[:, :, 0:QH],
                             gate[i][:, :, 0:QH])
        nc.vector.tensor_add(fus[:, :, 0:QH], fus[:, :, 0:QH], e[:, :, 0:QH])
        for u in range(2):
            finish_out_qh(i, u, 0)
        nc.vector.reciprocal(rp[:, 512:QS], m[:, 128:192])
        for ch, ups in ((0, u0), (1, u1)):
            nc.vector.tensor_mul(e[:, ch, QH:512], ups[:, QH:512],
                                 rp[:, QH:512])
            nc.vector.tensor_mul(e[:, ch, 512:QS],
                                 m[:, ch * 64:(ch + 1) * 64], rp[:, 512:QS])
        nc.vector.tensor_mul(e[:, :, QH:QS], e[:, :, QH:QS],
                             gate[i][:, :, QH:QS])
        nc.vector.tensor_add(fus[:, :, QH:QS], fus[:, :, QH:QS],
                             e[:, :, QH:QS])
        for u in range(2):
            finish_out_qh(i, u, 1)

    def finish_out_qh(i, u, qh):
        fus = fus_t[i]
        po = proj_ps()
        for t in range(2):
            nc.tensor.matmul(
                po[:, 0, 0:QH], wsb["out_wt"][:, t, u * P:(u + 1) * P],
                fus[:, t, qh * QH:(qh + 1) * QH],
                start=(t == 0), stop=(t == 1), skip_group_check=True)
        osb = osb_pool.tile([P, QH], F32, tag="osbq")
        nc.vector.tensor_copy(osb[:, :], po[:, 0, 0:QH])
        q = nc.sync if qh == 0 else nc.gpsimd
        q.dma_start(out=io["out"][i, u * P:(u + 1) * P,
                                  qh * QH:(qh + 1) * QH], in_=osb[:, :])

    def finish_out(i, u):
        # out = Wout fused; copy + 2-way chunked DMA out
        fus = fus_t[i]
        po = proj_ps()
        for qh in range(2):
            for t in range(2):
                nc.tensor.matmul(
                    po[:, qh, 0:QH], wsb["out_wt"][:, t, u * P:(u + 1) * P],
                    fus[:, t, qh * QH:(qh + 1) * QH],
                    start=(t == 0), stop=(t == 1), skip_group_check=True)
        osb = osb_pool.tile([P, QS], F32, tag="osb")
        nc.vector.tensor_copy(osb[:, :], po[:, :, 0:QH])
        nc.sync.dma_start(out=io["out"][i, u * P:(u + 1) * P, 0:QH],
                          in_=osb[:, 0:QH])
        nc.gpsimd.dma_start(out=io["out"][i, u * P:(u + 1) * P, QH:QS],
                            in_=osb[:, QH:QS])

    fus_t = {}

    NP = len(pairs)
    LASTI = pairs[NP - 1][1]

    def emit_gate_filler(k):
        if k < len(gate_units):
            gi, gu = gate_units[k]
            project_gate_unit(gi, gu)

    def alloc_state(p):
        state[p] = (
            ps_u0.tile([P, 512], F32, tag="u0", name="u0"),
            ps_u1.tile([P, 512], F32, tag="u1", name="u1"),
            ps_d.tile([P, 512], F32, tag="d", name="d"),
            ps_m.tile([P, 512], F32, tag="m", name="m"),
        )

    for slot in range(NP):
        p_s = pairs[slot]                                  # scores phase
        p_v = pairs[slot - 1] if slot >= 1 else None       # PV phase
        p_n = pairs[slot - 2] if slot >= 2 else None       # deferred norm
        j, i = p_s
        pt_t[p_s] = pt_pool.tile([P, KC2, 2, QS], FP8, tag="pt",
                                 name=f"pt{j}{i}")
        if i == 0:
            vst_t[j] = vst_pool.tile([P, KC2, 2, C], FP8, tag="vst",
                                     name=f"vst{j}")
        for mtick in range(KC2):
            s_unit(p_s, 2 * mtick)
            if p_n is not None and mtick == 0:
                normalize_free_m(p_n)
            if p_v is not None and mtick >= 1:
                if mtick == 1:
                    alloc_state(p_v)
                u_unit(p_v, mtick - 1)
                if mtick <= 4:
                    d_unit(p_v, 2 * (mtick - 1))
            s_unit(p_s, 2 * mtick + 1)
            if slot == 0:
                emit_gate_filler(mtick)
            if p_n is not None and mtick == 2:
                normalize_products(p_n, first=(p_n[0] == 0))
                if DEBUG_DUMP:
                    nc.sync.dma_start(
                        out=io["dbg_ctx"][p_n[0] * 3 + p_n[1]],
                        in_=ctxa[p_n[1]][:, :, :])
            if p_n is not None and p_n[0] == 2:
                if mtick == 3:
                    finish_blend(p_n[1])
                elif mtick == 5:
                    finish_out(p_n[1], 0)
                elif mtick == 6:
                    finish_out(p_n[1], 1)
            if p_v is not None:
                if mtick <= 4 and mtick >= 1:
                    d_unit(p_v, 2 * mtick - 1)
                elif mtick == 5:
                    d_unit(p_v, KC2 - 1)
                    recip_pair(p_v)
        if p_v is not None:
            u_unit(p_v, KC2 - 1)
            normalize_free(p_v)
        if DEBUG_DUMP and p_s == (0, 0):
            nc.sync.dma_start(out=io["dbg_pt"][:], in_=pt_t[p_s][:, :, :, :])
            nc.sync.dma_start(out=io["dbg_vst"][:], in_=vst_t[0][:, :, :, :])

    # ---- epilogue: PV of the last pair, hand-interleaved with the
    # previous pair's normalize/finish so the PE queue never blocks.
    p_v, p_n = pairs[NP - 1], pairs[NP - 2]
    alloc_state(p_v)
    normalize_free_m(p_n)
    for kc2 in range(3):
        u_unit(p_v, kc2)
        d_unit(p_v, 2 * kc2)
        d_unit(p_v, 2 * kc2 + 1)
    normalize_products(p_n, first=False)
    if DEBUG_DUMP:
        nc.sync.dma_start(out=io["dbg_ctx"][p_n[0] * 3 + p_n[1]],
                          in_=ctxa[p_n[1]][:, :, :])
    finish_blend(p_n[1])
    for kc2 in range(3, 6):
        u_unit(p_v, kc2)
        for dk in (2 * kc2, 2 * kc2 + 1):
            if dk < KC2:
                d_unit(p_v, dk)
    recip_pair(p_v)
    finish_blend_pre(LASTI)
    finish_out(p_n[1], 0)
    for kc2 in range(6, KC2):
        u_unit(p_v, kc2)
    finish_out(p_n[1], 1)
    finish_last(p_v)
    if DEBUG_DUMP:
        nc.sync.dma_start(out=io["dbg_ctx"][8], in_=ctxa[2][:, :, :])
    finish_out(LASTI, 0)
    finish_out(LASTI, 1)

def _build():
    nc = bacc.Bacc("TRN2", target_bir_lowering=False, debug=False,
                   num_devices=NCORES)
    io = {}
    for j in range(3):
        io[f"x8{j}"] = nc.declare_dram_parameter(f"x8{j}", [128, KC, 2 * 128],
                                                 FP8, isOutput=False)
        io[f"xq{j}"] = nc.declare_dram_parameter(f"xq{j}", [128, 2, QS], BF16,
                                                 isOutput=False)
    for wn, dt_ in (("m_t", BF16), ("v_wt8", FP8), ("v_lo8", FP8),
                    ("gate_wt", BF16), ("out_wt", BF16)):
        io[wn] = nc.declare_dram_parameter(wn, [128, 2, C], dt_, isOutput=False)
    io["out"] = nc.declare_dram_parameter("out", [3, C, QS], F32, isOutput=True)
    if DEBUG_DUMP:
        io["dbg_qg"] = nc.declare_dram_parameter(
            "dbg_qg", [3, 128, 2, QS], FP8, isOutput=True)
        io["dbg_gate"] = nc.declare_dram_parameter(
            "dbg_gate", [3, 128, 2, QS], BF16, isOutput=True)
        io["dbg_pt"] = nc.declare_dram_parameter(
            "dbg_pt", [128, KC2, 2, QS], FP8, isOutput=True)
        io["dbg_vst"] = nc.declare_dram_parameter(
            "dbg_vst", [128, KC2, 2, C], FP8, isOutput=True)
        io["dbg_ctx"] = nc.declare_dram_parameter(
            "dbg_ctx", [9, 128, 2, QS], F32, isOutput=True)

    with tile.TileContext(nc) as tc:
        with ExitStack() as ctx:
            _emit(ctx, tc, io)
    nc.compile()
    return nc


_CACHED_NC = None


def _get_nc():
    global _CACHED_NC
    if _CACHED_NC is None:
        _CACHED_NC = _build()
    return _CACHED_NC


def _prep_host(inputs: dict):
    f32 = np.float32
    bf16 = ml_dtypes.bfloat16
    f8 = ml_dtypes.float8_e4m3
    feats = [np.ascontiguousarray(inputs[f"feat{j}"], dtype=f32).reshape(B, C, N)
             for j in range(3)]
    for bn in ("q_b", "k_b", "v_b", "gate_b", "out_b"):
        if not np.all(np.asarray(inputs[bn]) == 0):
            raise NotImplementedError(f"{bn} != 0 unsupported (spec fill=zeros)")
    pos = np.asarray(inputs["pos_embedding"], f32).reshape(C, 1)
    scale = np.float32(C ** -0.5)

    def cpt(a, dt_):  # [C, X] -> [128, 2, X] with c = t*128 + p
        return np.ascontiguousarray(
            a.reshape(2, 128, a.shape[1]).transpose(1, 0, 2).astype(dt_))

    q_w = np.asarray(inputs["q_w"], f32)
    k_w = np.asarray(inputs["k_w"], f32)
    v_s = np.asarray(inputs["v_w"], f32).T * np.float32(VS)
    v_hi = v_s.astype(f8)
    shared = {
        "m_t": cpt((q_w.T @ k_w) * scale, bf16),
        "v_wt8": cpt(v_hi.astype(f32), f8),
        "v_lo8": cpt(v_s - v_hi.astype(f32), f8),
        "gate_wt": cpt(np.asarray(inputs["gate_w"], f32).T, bf16),
        "out_wt": cpt(np.asarray(inputs["out_w"], f32).T, bf16),
    }
    def interleave_x8(xall):
        # [C, N] -> [128, KC, 256] fp8: per 128-key chunk, pair-interleave
        # the two 128-channel planes with keys reversed (the SwInterleave
        # stationary layout; the matmul output key order comes out normal).
        a = xall.reshape(2, 128, KC, 128)        # [t, p, kc, k]
        a = a[:, :, :, ::-1].transpose(1, 2, 3, 0)   # [p, kc, k_rev, t]
        return np.ascontiguousarray(a.reshape(128, KC, 256).astype(f8))

    in_maps = []
    for r in range(NCORES):
        b, s = r // NSLICE, r % NSLICE
        im = dict(shared)
        for j in range(3):
            xall = feats[j][b] + pos
            im[f"x8{j}"] = interleave_x8(xall)
            im[f"xq{j}"] = cpt(xall[:, s * QS:(s + 1) * QS], bf16)
        in_maps.append(im)
    return in_maps


def _run(inputs: dict, trace: bool = False, tmpdir: str | None = None):
    in_maps = _prep_host(inputs)
    nc = _get_nc()
    res = run_bass_kernel_spmd(nc, in_maps, core_ids=list(range(NCORES)),
                               trace=trace, tmpdir=tmpdir)
    full = np.empty((3, B, C, N), dtype=np.float32)
    for r in range(NCORES):
        b, s = r // NSLICE, r % NSLICE
        full[:, b, :, s * QS:(s + 1) * QS] = res.results[r]["out"]
    full = full.reshape(3, B, C, H, W)
    return (full[0], full[1], full[2]), res


def kernel(**inputs):
    outs, _ = _run(inputs, trace=bool(os.environ.get("KERNEL_TRACE")))
    return outs
